# revision 2
# baseline (speedup 1.0000x reference)
"""Channelwise tensor product (e3nn-style) GNN message passing on 8 TRN2 cores.

kernel(**inputs) takes the full (unsharded) problem and returns
(out0, out1) matching the reference:
    out0: (num_nodes, 64, 1) f32,  out1: (num_nodes, 64, 3) f32

Strategy (per the sharding hint: partition edges, replicate node features):
 - Host planning: bucket edges by receiver (core = receiver // 6250, then
   128-node output tile within the core), split each bucket into lo/hi
   sender groups (sender < 32768) so the int16-indexed dma_gather custom op
   can fetch x1[sender] rows, pad each group to 128-edge tiles with
   zero-weight edges, and prefold x2_0e and the CG normalization into the
   per-edge weights (bf16).
 - Device (SPMD, identical program on all 8 cores, no collectives needed --
   receiver ranges are disjoint): stream prefolded weights, dma_gather
   x1 rows (bf16, 4 SWDGE queues in parallel), build the four tensor
   product paths on DVE/ACT with a k-major layout for the 1o irreps so the
   bf16 2x DVE mode engages, build one-hot receiver selection matrices via
   is_equal against a constant iota, and segment-sum via PE matmul
   accumulation into one PSUM tile per 128-node bucket, each written once.
"""
import numpy as np
import ml_dtypes
from contextlib import ExitStack

import concourse.bass as bass
import concourse.bacc as bacc
import concourse.tile as tile
from concourse import mybir
from concourse.bass_utils import run_bass_kernel_spmd

BF16 = mybir.dt.bfloat16
F32 = mybir.dt.float32
I16 = mybir.dt.int16
NPBF = ml_dtypes.bfloat16

NCORES = 8
MUL = 32
VLO_CAP = 32768
PAIR = 1          # buckets per gather group
INV_SQRT3 = np.float32(1.0 / np.sqrt(3.0, dtype=np.float32))


def plan_and_shard(weights, x1_0e, x1_1o, x2_0e, x2_1o, senders, receivers,
                   num_nodes):
    E = weights.shape[0]
    N = int(num_nodes)
    NLOC = (N + NCORES - 1) // NCORES
    NTB = (NLOC + 127) // 128
    VLO = min(VLO_CAP, N)

    senders = np.asarray(senders).astype(np.int64)
    receivers = np.asarray(receivers).astype(np.int64)
    w = np.asarray(weights, dtype=np.float32).reshape(E, 4, MUL)
    sh0 = np.asarray(x2_0e, dtype=np.float32).reshape(E, 1)
    sh1 = np.asarray(x2_1o, dtype=np.float32).reshape(E, 3)

    core = np.minimum(receivers // NLOC, NCORES - 1)
    rloc = receivers - core * NLOC
    bucket = rloc >> 7
    hi = (senders >= VLO).astype(np.int64)

    key = (core * NTB + bucket) * 2 + hi
    ngroups = NCORES * NTB * 2
    counts = np.bincount(key, minlength=ngroups).reshape(NCORES, NTB, 2)

    T_lo = np.maximum((counts[:, :, 0].max(axis=0) + 127) // 128, 1)
    T_hi = np.maximum((counts[:, :, 1].max(axis=0) + 127) // 128, 1)

    # groups of PAIR buckets; tile order per group: lo(b0) lo(b1) hi(b0) hi(b1)
    NG = (NTB + PAIR - 1) // PAIR
    groups = [list(range(g * PAIR, min((g + 1) * PAIR, NTB))) for g in range(NG)]
    base_lo = np.zeros(NTB, np.int64)   # edge slot base per (bucket, half)
    base_hi = np.zeros(NTB, np.int64)
    gmeta = []                          # per group: (t0, Tlo_g, Thi_g, buckets)
    t = 0
    for bs in groups:
        t0 = t
        for b in bs:
            base_lo[b] = t * 128
            t += int(T_lo[b])
        for b in bs:
            base_hi[b] = t * 128
            t += int(T_hi[b])
        gmeta.append((t0, int(sum(T_lo[b] for b in bs)),
                      int(sum(T_hi[b] for b in bs)), list(bs)))
    TT = t
    EP = TT * 128

    order = np.argsort(key, kind='stable')
    sorted_key = key[order]
    grp_start = np.searchsorted(sorted_key, np.arange(ngroups), side='left')
    rank = np.empty(E, np.int64)
    rank[order] = np.arange(E) - grp_start[sorted_key]
    slot = np.where(hi == 0, base_lo[bucket], base_hi[bucket]) + rank

    wcat = np.empty((E, 128), dtype=np.float32)
    wcat[:, 0:32] = w[:, 0] * sh0
    wcat[:, 32:64] = w[:, 1]
    wcat[:, 64:96] = w[:, 2] * sh0
    wcat[:, 96:128] = w[:, 3] * INV_SQRT3

    x1cat = np.empty((N, 128), dtype=np.float32)
    x1cat[:, 0:32] = np.asarray(x1_0e, np.float32).reshape(N, MUL)
    x1cat[:, 32:128] = np.asarray(x1_1o, np.float32).reshape(N, MUL, 3) \
        .transpose(0, 2, 1).reshape(N, 96)
    x1cat = x1cat.astype(NPBF)

    iota = np.tile(np.arange(128, dtype=np.float32).astype(NPBF)[None, :],
                   (128, 1)).copy()

    in_maps = []
    sidx_local = np.where(hi == 0, senders, senders - VLO).astype(np.int16)
    recv_rel = (rloc - (bucket << 7)).astype(np.float32)
    for k in range(NCORES):
        m = core == k
        sl = slot[m]
        wpad = np.zeros((EP, 128), dtype=NPBF)
        wpad[sl] = wcat[m].astype(NPBF)
        sh1pad = np.zeros((EP, 3), dtype=NPBF)
        sh1pad[sl] = sh1[m].astype(NPBF)
        rrpad = np.zeros(EP, dtype=np.float32)
        rrpad[sl] = recv_rel[m]
        sipad = np.zeros(EP, dtype=np.int16)
        sipad[sl] = sidx_local[m]

        wdev = wpad.reshape(TT, 128, 128).transpose(1, 0, 2) \
            .reshape(128, TT * 128).copy()
        sh1dev = sh1pad.reshape(TT, 128, 3).transpose(1, 0, 2) \
            .reshape(128, TT * 3).copy()
        rrdev = rrpad.reshape(TT, 128).T.astype(NPBF).copy()

        idx = np.zeros((128, TT * 8), dtype=np.int16)
        for gn, (t0, Tlo_g, Thi_g, bs) in enumerate(gmeta):
            for qn, Tg, toff in ((gn % 4, Tlo_g, t0),
                                 ((gn + 2) % 4, Thi_g, t0 + Tlo_g)):
                n = Tg * 128
                blk = sipad[toff * 128:toff * 128 + n]
                g = np.arange(n)
                pb = 32 * qn
                idx[pb + g % 16, toff * 8 + g // 16] = blk
                idx[pb + 16 + g % 16, toff * 8 + g // 16] = blk

        in_maps.append({
            "wdev": wdev, "sh1dev": sh1dev, "rrdev": rrdev, "idx": idx,
            "x1cat": x1cat, "iota": iota,
        })

    meta = dict(N=N, NLOC=NLOC, NTB=NTB, VLO=VLO, TT=TT,
                T_lo=[int(v) for v in T_lo], T_hi=[int(v) for v in T_hi],
                gmeta=gmeta)
    return meta, in_maps


def build_program(meta):
    N, NTB, VLO, TT = meta["N"], meta["NTB"], meta["VLO"], meta["TT"]
    T_lo, T_hi, gmeta = meta["T_lo"], meta["T_hi"], meta["gmeta"]

    nc = bacc.Bacc("TRN2", target_bir_lowering=False, debug=False,
                   num_devices=NCORES, dynamic_dma_scratch_size=65536,
                   num_swdge_queues=4)
    wdev_d = nc.dram_tensor("wdev", [128, TT * 128], BF16, kind="ExternalInput").ap()
    sh1_d = nc.dram_tensor("sh1dev", [128, TT * 3], BF16, kind="ExternalInput").ap()
    rr_d = nc.dram_tensor("rrdev", [128, TT], BF16, kind="ExternalInput").ap()
    idx_d = nc.dram_tensor("idx", [128, TT * 8], I16, kind="ExternalInput").ap()
    x1_d = nc.dram_tensor("x1cat", [N, 128], BF16, kind="ExternalInput").ap()
    iota_d = nc.dram_tensor("iota", [128, 128], BF16, kind="ExternalInput").ap()
    out_d = nc.dram_tensor("out", [NTB, 128, 256], F32, kind="ExternalOutput").ap()

    mm = mybir.AluOpType.mult
    with tile.TileContext(nc) as tc:
        with ExitStack() as ctx:
            cpool = ctx.enter_context(tc.tile_pool(name="const", bufs=1))
            gpool = ctx.enter_context(tc.tile_pool(name="gath", bufs=8))
            wpool = ctx.enter_context(tc.tile_pool(name="wts", bufs=2))
            epool = ctx.enter_context(tc.tile_pool(name="exp", bufs=2))
            tpool = ctx.enter_context(tc.tile_pool(name="tmp", bufs=2))
            mpool = ctx.enter_context(tc.tile_pool(name="msg", bufs=3))
            spool = ctx.enter_context(tc.tile_pool(name="sel", bufs=2))
            ppool = ctx.enter_context(tc.tile_pool(name="psum", bufs=4, space="PSUM"))
            opool = ctx.enter_context(tc.tile_pool(name="outs", bufs=2))

            idx_sb = cpool.tile([128, TT * 8], I16)
            nc.sync.dma_start(idx_sb[:], idx_d[:])
            rr_sb = cpool.tile([128, TT], BF16)
            nc.sync.dma_start(rr_sb[:], rr_d[:])
            sh1_sb = cpool.tile([128, TT * 3], BF16)
            nc.sync.dma_start(sh1_sb[:], sh1_d[:])
            iota_sb = cpool.tile([128, 128], BF16)
            nc.sync.dma_start(iota_sb[:], iota_d[:])

            for gn, (t0, Tlg, Thg, bs) in enumerate(gmeta):
                T = Tlg + Thg
                g = gpool.tile([128, T, 128], BF16, tag="g")
                nc.gpsimd.dma_gather(
                    g[:, 0:Tlg, :], x1_d[0:VLO, :],
                    idx_sb[:, t0 * 8:(t0 + Tlg) * 8],
                    num_idxs=Tlg * 128, num_idxs_reg=Tlg * 128, elem_size=128,
                    single_packet=False, queue_num=gn % 4)
                nc.gpsimd.dma_gather(
                    g[:, Tlg:T, :], x1_d[VLO:N, :],
                    idx_sb[:, (t0 + Tlg) * 8:(t0 + T) * 8],
                    num_idxs=Thg * 128, num_idxs_reg=Thg * 128, elem_size=128,
                    single_packet=False, queue_num=(gn + 2) % 4)
                w = wpool.tile([128, T, 128], BF16, tag="w")
                nc.sync.dma_start(
                    w[:], wdev_d[:, t0 * 128:(t0 + T) * 128]
                    .rearrange("p (t f) -> p t f", f=128))

                s0 = g[:, :, 0:32]
                s1 = g[:, :, 32:128].rearrange("p t (k c) -> p t k c", c=32)
                A0 = w[:, :, 0:32]
                w1 = w[:, :, 32:64]
                A2 = w[:, :, 64:96]
                w3 = w[:, :, 96:128]

                she = epool.tile([128, T, 3, 32], BF16, tag="she")
                nc.scalar.copy(
                    she[:],
                    sh1_sb[:, t0 * 3:(t0 + T) * 3]
                    .rearrange("p (t k) -> p t k", k=3)
                    .unsqueeze(3).to_broadcast([128, T, 3, 32]))

                msg = mpool.tile([128, T, 256], BF16, tag="msg")
                nc.vector.tensor_tensor(out=msg[:, :, 0:32], in0=A0, in1=s0, op=mm)
                t3 = tpool.tile([128, T, 3, 32], BF16, tag="t3")
                nc.vector.tensor_tensor(out=t3[:], in0=s1, in1=she[:], op=mm)
                dot = tpool.tile([128, T, 32], BF16, tag="dot")
                nc.vector.tensor_add(out=dot[:], in0=t3[:, :, 0, :], in1=t3[:, :, 1, :])
                nc.vector.tensor_add(out=dot[:], in0=dot[:], in1=t3[:, :, 2, :])
                nc.vector.tensor_tensor(out=msg[:, :, 32:64], in0=dot[:], in1=w3, op=mm)
                t1 = tpool.tile([128, T, 32], BF16, tag="t1")
                nc.vector.tensor_tensor(out=t1[:], in0=w1, in1=s0, op=mm)
                nc.vector.tensor_tensor(
                    out=msg[:, :, 64:160].rearrange("p t (k c) -> p t k c", c=32),
                    in0=t1[:].unsqueeze(2).to_broadcast([128, T, 3, 32]),
                    in1=she[:], op=mm)
                nc.vector.tensor_tensor(
                    out=msg[:, :, 160:256].rearrange("p t (k c) -> p t k c", c=32),
                    in0=A2.unsqueeze(2).to_broadcast([128, T, 3, 32]),
                    in1=s1, op=mm)

                S = spool.tile([128, T, 128], BF16, tag="S")
                nc.vector.tensor_tensor(
                    out=S[:],
                    in0=iota_sb[:].unsqueeze(1).to_broadcast([128, T, 128]),
                    in1=rr_sb[:, t0:t0 + T].unsqueeze(2)
                    .to_broadcast([128, T, 128]),
                    op=mybir.AluOpType.is_equal)

                # per-bucket tile ranges within the group (lo run + hi run)
                loff = 0
                hoff = Tlg
                for b in bs:
                    ps = ppool.tile([128, 256], F32, tag="ps")
                    tiles = list(range(loff, loff + T_lo[b])) + \
                        list(range(hoff, hoff + T_hi[b]))
                    loff += T_lo[b]
                    hoff += T_hi[b]
                    for j, t in enumerate(tiles):
                        nc.tensor.matmul(ps[:], lhsT=S[:, t, :], rhs=msg[:, t, :],
                                         start=(j == 0), stop=(j == len(tiles) - 1),
                                         skip_group_check=True)
                    ob = opool.tile([128, 256], F32, tag="ob")
                    nc.scalar.copy(ob[:], ps[:])
                    nc.sync.dma_start(out_d[b], ob[:])
    nc.compile()
    return nc


def postprocess(meta, results):
    N, NLOC, NTB = meta["N"], meta["NLOC"], meta["NTB"]
    outs = []
    for k in range(NCORES):
        o = results[k]["out"].reshape(NTB * 128, 256)
        lo = k * NLOC
        outs.append(o[:min(NLOC, N - lo)])
    o = np.concatenate(outs, axis=0)
    out0 = np.ascontiguousarray(o[:, 0:64]).reshape(N, 64, 1).astype(np.float32)
    m1a = o[:, 64:160].reshape(N, 3, 32).transpose(0, 2, 1)
    m1b = o[:, 160:256].reshape(N, 3, 32).transpose(0, 2, 1)
    out1 = np.ascontiguousarray(np.concatenate([m1a, m1b], axis=1)).astype(np.float32)
    return out0, out1


def kernel(weights, x1_0e, x1_1o, x2_0e, x2_1o, senders, receivers, num_nodes,
           trace=False, tmpdir=None):
    meta, in_maps = plan_and_shard(weights, x1_0e, x1_1o, x2_0e, x2_1o,
                                   senders, receivers, num_nodes)
    nc = build_program(meta)
    res = run_bass_kernel_spmd(nc, in_maps, list(range(NCORES)),
                               trace=trace, tmpdir=tmpdir)
    out = postprocess(meta, res.results)
    return out, res


# revision 3
# speedup vs baseline: 1.0302x; 1.0302x over previous
"""Channelwise tensor product (e3nn-style) GNN message passing on 8 TRN2 cores.

kernel(**inputs) takes the full (unsharded) problem and returns
(out0, out1) matching the reference:
    out0: (num_nodes, 64, 1) f32,  out1: (num_nodes, 64, 3) f32

Strategy (per the sharding hint: partition edges, replicate node features):
 - Host planning: bucket edges by receiver (core = receiver // 6250, then
   128-node output tile within the core), split each bucket into lo/hi
   sender groups (sender < 32768) so the int16-indexed dma_gather custom op
   can fetch x1[sender] rows, pad each group to 128-edge tiles with
   zero-weight edges, and prefold x2_0e and the CG normalization into the
   per-edge weights (bf16).
 - Device (SPMD, identical program on all 8 cores, no collectives needed --
   receiver ranges are disjoint): stream prefolded weights, dma_gather
   x1 rows (bf16, 4 SWDGE queues in parallel), build the four tensor
   product paths on DVE/ACT with a k-major layout for the 1o irreps so the
   bf16 2x DVE mode engages, build one-hot receiver selection matrices via
   is_equal against a constant iota, and segment-sum via PE matmul
   accumulation into one PSUM tile per 128-node bucket, each written once.
"""
import numpy as np
import ml_dtypes
from contextlib import ExitStack

import concourse.bass as bass
import concourse.bacc as bacc
import concourse.tile as tile
from concourse import mybir
from concourse.bass_utils import run_bass_kernel_spmd

BF16 = mybir.dt.bfloat16
F32 = mybir.dt.float32
I16 = mybir.dt.int16
NPBF = ml_dtypes.bfloat16

NCORES = 8
MUL = 32
VLO_CAP = 32768
PAIR = 1          # buckets per gather group
INV_SQRT3 = np.float32(1.0 / np.sqrt(3.0, dtype=np.float32))


def plan_and_shard(weights, x1_0e, x1_1o, x2_0e, x2_1o, senders, receivers,
                   num_nodes):
    E = weights.shape[0]
    N = int(num_nodes)
    NLOC = (N + NCORES - 1) // NCORES
    NTB = (NLOC + 127) // 128
    VLO = min(VLO_CAP, N)

    senders = np.asarray(senders).astype(np.int64)
    receivers = np.asarray(receivers).astype(np.int64)
    w = np.asarray(weights, dtype=np.float32).reshape(E, 4, MUL)
    sh0 = np.asarray(x2_0e, dtype=np.float32).reshape(E, 1)
    sh1 = np.asarray(x2_1o, dtype=np.float32).reshape(E, 3)

    core = np.minimum(receivers // NLOC, NCORES - 1)
    rloc = receivers - core * NLOC
    bucket = rloc >> 7
    hi = (senders >= VLO).astype(np.int64)

    key = (core * NTB + bucket) * 2 + hi
    ngroups = NCORES * NTB * 2
    counts = np.bincount(key, minlength=ngroups).reshape(NCORES, NTB, 2)

    T_lo = np.maximum((counts[:, :, 0].max(axis=0) + 127) // 128, 1)
    T_hi = np.maximum((counts[:, :, 1].max(axis=0) + 127) // 128, 1)

    # groups of PAIR buckets; tile order per group: lo(b0) lo(b1) hi(b0) hi(b1)
    NG = (NTB + PAIR - 1) // PAIR
    groups = [list(range(g * PAIR, min((g + 1) * PAIR, NTB))) for g in range(NG)]
    base_lo = np.zeros(NTB, np.int64)   # edge slot base per (bucket, half)
    base_hi = np.zeros(NTB, np.int64)
    gmeta = []                          # per group: (t0, Tlo_g, Thi_g, buckets)
    t = 0
    for bs in groups:
        t0 = t
        for b in bs:
            base_lo[b] = t * 128
            t += int(T_lo[b])
        for b in bs:
            base_hi[b] = t * 128
            t += int(T_hi[b])
        gmeta.append((t0, int(sum(T_lo[b] for b in bs)),
                      int(sum(T_hi[b] for b in bs)), list(bs)))
    TT = t
    EP = TT * 128

    order = np.argsort(key, kind='stable')
    sorted_key = key[order]
    grp_start = np.searchsorted(sorted_key, np.arange(ngroups), side='left')
    rank = np.empty(E, np.int64)
    rank[order] = np.arange(E) - grp_start[sorted_key]
    slot = np.where(hi == 0, base_lo[bucket], base_hi[bucket]) + rank

    wcat = np.empty((E, 128), dtype=np.float32)
    wcat[:, 0:32] = w[:, 0] * sh0
    wcat[:, 32:64] = w[:, 1]
    wcat[:, 64:96] = w[:, 2] * sh0
    wcat[:, 96:128] = w[:, 3] * INV_SQRT3

    x1cat = np.empty((N, 128), dtype=np.float32)
    x1cat[:, 0:32] = np.asarray(x1_0e, np.float32).reshape(N, MUL)
    x1cat[:, 32:128] = np.asarray(x1_1o, np.float32).reshape(N, MUL, 3) \
        .transpose(0, 2, 1).reshape(N, 96)
    x1cat = x1cat.astype(NPBF)

    iota = np.tile(np.arange(128, dtype=np.float32).astype(NPBF)[None, :],
                   (128, 1)).copy()

    in_maps = []
    sidx_local = np.where(hi == 0, senders, senders - VLO).astype(np.int16)
    recv_rel = (rloc - (bucket << 7)).astype(np.float32)
    for k in range(NCORES):
        m = core == k
        sl = slot[m]
        wpad = np.zeros((EP, 128), dtype=NPBF)
        wpad[sl] = wcat[m].astype(NPBF)
        sh1pad = np.zeros((EP, 3), dtype=NPBF)
        sh1pad[sl] = sh1[m].astype(NPBF)
        rrpad = np.zeros(EP, dtype=np.float32)
        rrpad[sl] = recv_rel[m]
        sipad = np.zeros(EP, dtype=np.int16)
        sipad[sl] = sidx_local[m]

        wdev = wpad.reshape(TT, 128, 128).transpose(1, 0, 2) \
            .reshape(128, TT * 128).copy()
        sh1dev = sh1pad.reshape(TT, 128, 3).transpose(1, 0, 2) \
            .reshape(128, TT * 3).copy()
        rrdev = rrpad.reshape(TT, 128).T.astype(NPBF).copy()

        idx = np.zeros((128, TT * 8), dtype=np.int16)
        for gn, (t0, Tlo_g, Thi_g, bs) in enumerate(gmeta):
            for qn, Tg, toff in ((gn % 4, Tlo_g, t0),
                                 ((gn + 2) % 4, Thi_g, t0 + Tlo_g)):
                n = Tg * 128
                blk = sipad[toff * 128:toff * 128 + n]
                g = np.arange(n)
                pb = 32 * qn
                idx[pb + g % 16, toff * 8 + g // 16] = blk
                idx[pb + 16 + g % 16, toff * 8 + g // 16] = blk

        in_maps.append({
            "wdev": wdev, "sh1dev": sh1dev, "rrdev": rrdev, "idx": idx,
            "x1cat": x1cat, "iota": iota,
        })

    meta = dict(N=N, NLOC=NLOC, NTB=NTB, VLO=VLO, TT=TT,
                T_lo=[int(v) for v in T_lo], T_hi=[int(v) for v in T_hi],
                gmeta=gmeta)
    return meta, in_maps


def build_program(meta):
    N, NTB, VLO, TT = meta["N"], meta["NTB"], meta["VLO"], meta["TT"]
    T_lo, T_hi, gmeta = meta["T_lo"], meta["T_hi"], meta["gmeta"]

    nc = bacc.Bacc("TRN2", target_bir_lowering=False, debug=False,
                   num_devices=NCORES, dynamic_dma_scratch_size=65536,
                   num_swdge_queues=4)
    wdev_d = nc.dram_tensor("wdev", [128, TT * 128], BF16, kind="ExternalInput").ap()
    sh1_d = nc.dram_tensor("sh1dev", [128, TT * 3], BF16, kind="ExternalInput").ap()
    rr_d = nc.dram_tensor("rrdev", [128, TT], BF16, kind="ExternalInput").ap()
    idx_d = nc.dram_tensor("idx", [128, TT * 8], I16, kind="ExternalInput").ap()
    x1_d = nc.dram_tensor("x1cat", [N, 128], BF16, kind="ExternalInput").ap()
    iota_d = nc.dram_tensor("iota", [128, 128], BF16, kind="ExternalInput").ap()
    out_d = nc.dram_tensor("out", [NTB, 128, 256], F32, kind="ExternalOutput").ap()

    mm = mybir.AluOpType.mult
    with tile.TileContext(nc) as tc:
        with ExitStack() as ctx:
            cpool = ctx.enter_context(tc.tile_pool(name="const", bufs=1))
            gpool = ctx.enter_context(tc.tile_pool(name="gath", bufs=8))
            wpool = ctx.enter_context(tc.tile_pool(name="wts", bufs=2))
            epool = ctx.enter_context(tc.tile_pool(name="exp", bufs=2))
            tpool = ctx.enter_context(tc.tile_pool(name="tmp", bufs=2))
            mpool = ctx.enter_context(tc.tile_pool(name="msg", bufs=3))
            spool = ctx.enter_context(tc.tile_pool(name="sel", bufs=2))
            ppool = ctx.enter_context(tc.tile_pool(name="psum", bufs=4, space="PSUM"))
            opool = ctx.enter_context(tc.tile_pool(name="outs", bufs=2))

            idx_sb = cpool.tile([128, TT * 8], I16)
            nc.sync.dma_start(idx_sb[:], idx_d[:])
            rr_sb = cpool.tile([128, TT], BF16)
            nc.sync.dma_start(rr_sb[:], rr_d[:])
            sh1_sb = cpool.tile([128, TT * 3], BF16)
            nc.sync.dma_start(sh1_sb[:], sh1_d[:])
            iota_sb = cpool.tile([128, 128], BF16)
            nc.sync.dma_start(iota_sb[:], iota_d[:])

            for gn, (t0, Tlg, Thg, bs) in enumerate(gmeta):
                T = Tlg + Thg
                g = gpool.tile([128, T, 128], BF16, tag="g")
                nc.gpsimd.dma_gather(
                    g[:, 0:Tlg, :], x1_d[0:VLO, :],
                    idx_sb[:, t0 * 8:(t0 + Tlg) * 8],
                    num_idxs=Tlg * 128, num_idxs_reg=Tlg * 128, elem_size=128,
                    single_packet=False, queue_num=gn % 4)
                nc.gpsimd.dma_gather(
                    g[:, Tlg:T, :], x1_d[VLO:N, :],
                    idx_sb[:, (t0 + Tlg) * 8:(t0 + T) * 8],
                    num_idxs=Thg * 128, num_idxs_reg=Thg * 128, elem_size=128,
                    single_packet=False, queue_num=(gn + 2) % 4)
                w = wpool.tile([128, T, 128], BF16, tag="w")
                nc.sync.dma_start(
                    w[:], wdev_d[:, t0 * 128:(t0 + T) * 128]
                    .rearrange("p (t f) -> p t f", f=128))

                s0 = g[:, :, 0:32]
                s1 = g[:, :, 32:128].rearrange("p t (k c) -> p t k c", c=32)
                A0 = w[:, :, 0:32]
                w1 = w[:, :, 32:64]
                A2 = w[:, :, 64:96]
                w3 = w[:, :, 96:128]

                she = epool.tile([128, T, 3, 32], BF16, tag="she")
                nc.scalar.copy(
                    she[:],
                    sh1_sb[:, t0 * 3:(t0 + T) * 3]
                    .rearrange("p (t k) -> p t k", k=3)
                    .unsqueeze(3).to_broadcast([128, T, 3, 32]))

                msg = mpool.tile([128, T, 256], BF16, tag="msg")
                nc.vector.tensor_tensor(out=msg[:, :, 0:32], in0=A0, in1=s0, op=mm)
                t3 = tpool.tile([128, T, 3, 32], BF16, tag="t3")
                nc.vector.tensor_tensor(out=t3[:], in0=s1, in1=she[:], op=mm)
                dot = tpool.tile([128, T, 32], BF16, tag="dot")
                nc.vector.tensor_add(out=dot[:], in0=t3[:, :, 0, :], in1=t3[:, :, 1, :])
                nc.vector.tensor_add(out=dot[:], in0=dot[:], in1=t3[:, :, 2, :])
                nc.vector.tensor_tensor(out=msg[:, :, 32:64], in0=dot[:], in1=w3, op=mm)
                t1 = tpool.tile([128, T, 32], BF16, tag="t1")
                nc.vector.tensor_tensor(out=t1[:], in0=w1, in1=s0, op=mm)
                nc.vector.tensor_tensor(
                    out=msg[:, :, 64:160].rearrange("p t (k c) -> p t k c", c=32),
                    in0=t1[:].unsqueeze(2).to_broadcast([128, T, 3, 32]),
                    in1=she[:], op=mm)
                nc.vector.tensor_tensor(
                    out=msg[:, :, 160:256].rearrange("p t (k c) -> p t k c", c=32),
                    in0=A2.unsqueeze(2).to_broadcast([128, T, 3, 32]),
                    in1=s1, op=mm)

                S = spool.tile([128, T, 128], BF16, tag="S")
                nc.vector.tensor_tensor(
                    out=S[:],
                    in0=iota_sb[:].unsqueeze(1).to_broadcast([128, T, 128]),
                    in1=rr_sb[:, t0:t0 + T].unsqueeze(2)
                    .to_broadcast([128, T, 128]),
                    op=mybir.AluOpType.is_equal)

                # per-bucket tile ranges within the group (lo run + hi run)
                loff = 0
                hoff = Tlg
                for b in bs:
                    ps = ppool.tile([128, 256], F32, tag="ps")
                    tiles = list(range(loff, loff + T_lo[b])) + \
                        list(range(hoff, hoff + T_hi[b]))
                    loff += T_lo[b]
                    hoff += T_hi[b]
                    for j, t in enumerate(tiles):
                        nc.tensor.matmul(ps[:], lhsT=S[:, t, :], rhs=msg[:, t, :],
                                         start=(j == 0), stop=(j == len(tiles) - 1),
                                         skip_group_check=True)
                    ob = opool.tile([128, 256], F32, tag="ob")
                    nc.scalar.copy(ob[:], ps[:])
                    nc.sync.dma_start(out_d[b], ob[:])
    nc.compile()
    return nc


def postprocess(meta, results):
    N, NLOC, NTB = meta["N"], meta["NLOC"], meta["NTB"]
    outs = []
    for k in range(NCORES):
        o = results[k]["out"].reshape(NTB * 128, 256)
        lo = k * NLOC
        outs.append(o[:min(NLOC, N - lo)])
    o = np.concatenate(outs, axis=0)
    out0 = np.ascontiguousarray(o[:, 0:64]).reshape(N, 64, 1).astype(np.float32)
    m1a = o[:, 64:160].reshape(N, 3, 32).transpose(0, 2, 1)
    m1b = o[:, 160:256].reshape(N, 3, 32).transpose(0, 2, 1)
    out1 = np.ascontiguousarray(np.concatenate([m1a, m1b], axis=1)).astype(np.float32)
    return out0, out1


def kernel(weights, x1_0e, x1_1o, x2_0e, x2_1o, senders, receivers, num_nodes,
           trace=False, tmpdir=None):
    meta, in_maps = plan_and_shard(weights, x1_0e, x1_1o, x2_0e, x2_1o,
                                   senders, receivers, num_nodes)
    nc = build_program(meta)
    res = run_bass_kernel_spmd(nc, in_maps, list(range(NCORES)),
                               trace=trace, tmpdir=tmpdir)
    out = postprocess(meta, res.results)
    if trace:
        return out, res
    return out
